# revision 36
# baseline (speedup 1.0000x reference)
"""TRN2 Bass kernel for nn_DecoderLayer_70781061038465 (Falcon-7B style decoder
layer: fractured LayerNorm -> parallel MQA attention + MLP -> residual).

Sharding: 8-way tensor parallelism, 9 head-slots per core (71 real + 1 pad),
MLP 4h split 2272/core. Each core emits a full-width partial of
16*(attn_out + mlp_out); the host sums partials, divides by 16, and adds the
residual.

Numerics plan (fits rel-err < 2e-2 with margin, measured 1.36e-2 in sim):
  - proj (q/k/v/4h) and MLP-down run in fp8e4 DoubleRow "tier-C": both weight
    and activation split hi+lo, computing Wh@xh + Wl@xh + Wh@xl (3 paired
    products per 2 k-tiles = 0.75 cycles/row vs bf16's 1.0).
  - dense (ct@wd) runs "tier-B": W hi+lo, ct single fp8 (0.5 cycles/row).
  - attention internals bf16. All fp8 tensors pre-scaled x16 into e4m3's
    normal range; the x16 is unwound via cos/sin tables (/16), activation
    scale= arguments, and a final /16 on the host reduction.

Layout plan (no on-device transposes of x): x is uploaded twice -- token-major
for LayerNorm stats (bn_stats) and hidden-major for the projection GEMM. The
per-token rstd is broadcast to a [128, T] row tile via tiny PE outer-products
and applied on the hidden-major tiles directly (LN mean/bias fold into extra
contraction rows as in the classic trick). Attention runs fully transposed
(scoresT straight off the PE, denominator as a 65th ones-column of V);
context returns to feature-major via PE-transposes. Phase-D partials go
PSUM -> DRAM by direct DMA.
"""
import sys
if "/opt/trn_rl_repo" not in sys.path:
    sys.path.insert(0, "/opt/trn_rl_repo")

from contextlib import ExitStack

import numpy as np
import ml_dtypes

import concourse.bass as bass
import concourse.tile as tile
from concourse import bacc, mybir
from concourse.bass_utils import run_bass_kernel_spmd

F32 = mybir.dt.float32
BF16 = mybir.dt.bfloat16
FP8 = mybir.dt.float8e4
AF = mybir.ActivationFunctionType
MUL = mybir.AluOpType.mult
SUB = mybir.AluOpType.subtract
ADD = mybir.AluOpType.add
DR = mybir.MatmulPerfMode.DoubleRow

# problem shapes (hardcoded per contract)
B, S, H, NH, HD = 2, 1024, 4544, 71, 64
T = B * S
HP = 4608                 # padded hidden (36*128)
KT = HP // 128            # 36 contraction k-tiles
NSLOT = 9                 # head slots per core (8*9 = 72 >= 71)
F4 = 4 * H
F4C = F4 // 8             # 2272 4h-features per core
GKT = 18                  # 4h k-tiles per core (18*128 = 2304)
CKT = 6                   # ct k-tiles (5 used: 10 slots * 64; k5 zero pad)
MT = 24                   # proj m-tiles: m0-4 q, m5 k|v, m6-23 4h
NHF = 18                  # phase-D output half-chunks of 256 (18*256 = 4608)
EPS = 1e-5
NEGM = -240.0             # causal mask pre-exp-scale (exp scale 1/8 -> -30)

_CACHE = {}


def _build():
    nc = bacc.Bacc("TRN2", target_bir_lowering=False, debug=False)
    xtd_d = nc.dram_tensor("xtd", [128, KT, T], BF16, kind="ExternalInput")
    xbd_d = nc.dram_tensor("xbd", [T, H], FP8, kind="ExternalInput")
    wpk_d = nc.dram_tensor("wpk", [128, MT, KT, 2, 128], FP8,
                           kind="ExternalInput")
    wdd_d = nc.dram_tensor("wdd", [128, NHF, 4, 12, 256], FP8,
                           kind="ExternalInput")
    csn_d = nc.dram_tensor("csn", [2, 128, S], BF16, kind="ExternalInput")
    dmk_d = nc.dram_tensor("dmk", [128, 128], F32, kind="ExternalInput")
    idn_d = nc.dram_tensor("idn", [128, 128], BF16, kind="ExternalInput")
    sel_d = nc.dram_tensor("seld", [8, 1024], FP8, kind="ExternalInput")
    out_d = nc.dram_tensor("out", [T, HP], BF16, kind="ExternalOutput")

    xtd = xtd_d.ap()
    xbd = xbd_d.ap()
    wpk = wpk_d.ap()
    wdd = wdd_d.ap()
    out = out_d.ap()

    with tile.TileContext(nc) as tc, ExitStack() as ctx:
        def pool(name, bufs, space="SBUF"):
            return ctx.enter_context(tc.tile_pool(name=name, bufs=bufs, space=space))

        const = pool("const", 1)
        xinp = pool("xinp", 2)        # token-major halves for stats
        smp = pool("smp", 1)          # rstd/mr gather + transpose
        rbcp = pool("rbc", 1)
        xrawp = pool("xraw", 2)       # hidden-major raw ring
        t2k = pool("t2k", 2)          # bf16 [128,1024] temps (xtmp / rot)
        xhlp = pool("xhl", 1)
        wpp = pool("wpp", 3)          # proj weight half-k tiles
        qtp = pool("qt", 1)
        ktp = pool("kt", 1)
        vtp = pool("vt", 1)
        vtm = pool("vtm", 1)
        gtm = pool("gtm", 1)
        cgcp = pool("cgc", 1)
        cggp = pool("cgg", 1)
        etbp = pool("etb", 2)
        c2p = pool("c2", 1)
        wdtp = pool("wdt", 4)
        obp = pool("ob", 2)
        small = pool("small", 2)
        psp = pool("psp", 8, space="PSUM")

        cos_sb = const.tile([128, S], BF16, tag="cos")
        nc.sync.dma_start(cos_sb[:], csn_d.ap()[0])
        sin_sb = const.tile([128, S], BF16, tag="sin")
        nc.sync.dma_start(sin_sb[:], csn_d.ap()[1])
        dmaskT = const.tile([128, 128], F32, tag="dmaskT")
        nc.sync.dma_start(dmaskT[:], dmk_d.ap())
        idn = const.tile([128, 128], BF16, tag="idn")
        nc.sync.dma_start(idn[:], idn_d.ap())
        # sel[r, rt*128+p] = 1 iff r == rt: K=8 matmul broadcasts smT row rt
        sel = const.tile([8, 1024], FP8, tag="sel")
        nc.sync.dma_start(sel[:], sel_d.ap())

        # persistent pads in xhl (written once; per-batch writes never touch)
        xhl = xhlp.tile([128, KT, 2, S], FP8, tag="xhl")
        nc.vector.memset(xhl[64:128, KT - 1, :, :], 0.0)
        nc.vector.memset(xhl[64:65, KT - 1, 0, :], 1.0)  # ones row (h=4544)

        cgc = cgcp.tile([128, CKT, S], FP8, tag="cgc")
        nc.vector.memset(cgc[:, 5, :], 0.0)              # dense pad k-tile
        nc.vector.memset(cgc[64:128, 4, :], 0.0)         # pad head slot 9

        cgg_by = {}

        def emit_A(b):
            # ================= Phase A: LN stats -> rstd broadcast =========
            sm = smp.tile([128, 128], BF16, tag="sm", name=f"sm_{b}")
            mv8 = small.tile([128, 8, 2], F32, tag="mv8", name=f"mv8_{b}")
            for rt in range(8):
                row0 = b * S + rt * 128
                st = small.tile([128, 12, 6], BF16, tag="st")
                for hh in range(4):
                    xin = xinp.tile([128, H // 4], FP8, tag=f"xin{hh % 2}")
                    (nc.sync if b == 0 else nc.gpsimd).dma_start(
                        xin[:], xbd[row0:row0 + 128,
                                    hh * (H // 4):(hh + 1) * (H // 4)])
                    nc.vector.bn_stats(st[:, hh * 3 + 0, :], xin[:, 0:512])
                    nc.vector.bn_stats(st[:, hh * 3 + 1, :], xin[:, 512:1024])
                    nc.vector.bn_stats(st[:, hh * 3 + 2, :], xin[:, 1024:1136])
                nc.vector.bn_aggr(mv8[:, rt, :], st[:])
            t1 = small.tile([128, 8], F32, tag="t1")
            nc.vector.tensor_scalar_add(t1[:], mv8[:, :, 1], EPS)
            nc.scalar.activation(t1[:], t1[:], AF.Sqrt)
            t3 = small.tile([128, 8], F32, tag="t3")
            nc.vector.reciprocal(t3[:], t1[:])
            nc.vector.tensor_copy(sm[:, 0:8], t3[:])
            t2 = small.tile([128, 8], F32, tag="t2")
            nc.vector.tensor_tensor(t2[:], mv8[:, :, 0], t3[:], op=MUL)
            nc.vector.tensor_scalar_mul(t2[:], t2[:], -16.0)
            nc.vector.tensor_copy(sm[:, 16:24], t2[:])
            smT = smp.tile([128, 128], BF16, tag="smT", name=f"smT_{b}")
            nc.sync.dma_start_transpose(smT[:], sm[:])
            rbc = rbcp.tile([128, S], BF16, tag="rbc", name=f"rbc_{b}")
            for rt in range(8):
                pso = psp.tile([128, 128], F32, tag="ps", name=f"pso_{b}_{rt}")
                nc.tensor.matmul(pso[:], sel[:, rt * 128:(rt + 1) * 128],
                                 smT[0:8, :], start=True, stop=True)
                nc.scalar.activation(rbc[:, rt * 128:(rt + 1) * 128], pso[:],
                                     AF.Copy)
            # mr contraction row (h=4545): fp8 hi/lo staged at partitions
            # 16..23, moved into partition 65 of xhl by DMA (engines cannot
            # address partition bases other than 0/32/64/96)
            smT8 = smp.tile([32, 128], FP8, tag="smT8", name=f"smT8_{b}")
            nc.vector.tensor_copy(smT8[:], smT[0:32, :])
            smT8l = smp.tile([32, 128], FP8, tag="smT8l", name=f"smT8l_{b}")
            nc.vector.tensor_tensor(smT8l[:], smT[0:32, :], smT8[:], op=SUB)
            nc.sync.dma_start(xhl[65:66, KT - 1, 0, :], smT8[16:24, :])
            nc.sync.dma_start(xhl[65:66, KT - 1, 1, :], smT8l[16:24, :])

            # ============ Phase A2: scale hidden-major x, split hi/lo ======
            for k in range(KT):
                nreal = 64 if k == KT - 1 else 128
                xr = xrawp.tile([128, S], BF16, tag="xr")
                (nc.sync if b == 0 else nc.gpsimd).dma_start(
                    xr[:], xtd[:, k, b * S:(b + 1) * S])
                nc.vector.tensor_tensor(xr[:nreal, :], xr[:nreal, :],
                                        rbc[:nreal, :], op=MUL)
                if b == 0:
                    nc.scalar.activation(xhl[:nreal, k, 0, :], xr[:nreal, :],
                                         AF.Copy)
                else:
                    # keep Act free for D0 evictions running concurrently
                    nc.vector.tensor_copy(xhl[:nreal, k, 0, :], xr[:nreal, :])
                eng = nc.vector if k % 2 == 0 else nc.gpsimd
                eng.tensor_tensor(xhl[:nreal, k, 1, :], xr[:nreal, :],
                                  xhl[:nreal, k, 0, :], op=SUB)

        def emit_BC(b):
            # ================= Phase B: fused projection ===================
            qt = qtp.tile([128, 5, S], BF16, tag="qt", name=f"qt_{b}")
            kt2 = ktp.tile([128, S], BF16, tag="kt2", name=f"kt2_{b}")
            vt = vtp.tile([128, 8, 65], BF16, tag="vt", name=f"vt_{b}")
            nc.vector.memset(vt[:, :, 64:65], 1.0)
            cgg = cggp.tile([128, GKT, 2, S], FP8, tag="cgg", name=f"cgg_{b}")
            cgg_by[b] = cgg
            # ============ Phase C: attention, staggered into B =============
            c2ref = [None]
            ets = {}

            def emit_scores(h, b=b, qt=qt, kt2=kt2):
                qb_ = (h % 2) * 64
                kb = qb_
                et = etbp.tile([128, 12, 512], BF16, tag="et",
                               name=f"et_{b}_{h}", bufs=2)
                ets[h] = et
                for sqc in range(2):
                    nsk = 4 if sqc == 0 else 8
                    e0 = 0 if sqc == 0 else 4
                    scols = slice(sqc * 512, sqc * 512 + 512)
                    for skt in range(nsk):
                        sp = psp.tile([128, 512], F32, tag="ps",
                                      name=f"sp_{b}_{h}_{sqc}_{skt}")
                        nc.tensor.matmul(
                            sp[:], kt2[kb:kb + 64, skt * 128:(skt + 1) * 128],
                            qt[qb_:qb_ + 64, h // 2, scols],
                            start=True, stop=True)
                        dg = skt - sqc * 4
                        if dg >= 0:
                            nc.vector.tensor_tensor(
                                sp[:, dg * 128:(dg + 1) * 128],
                                sp[:, dg * 128:(dg + 1) * 128],
                                dmaskT[:], op=ADD)
                        nc.scalar.activation(et[:, e0 + skt, :], sp[:], AF.Exp,
                                             scale=0.125)

            def emit_ctx(h, b=b, c2ref=c2ref, vt=vt):
                qb_ = (h % 2) * 64
                if h % 2 == 0:
                    c2ref[0] = c2p.tile([128, 8, 128], BF16, tag="c2",
                                        name=f"c2_{b}_{h}")
                    if h == 8:
                        nc.vector.memset(c2ref[0][:, :, 64:128], 0.0)
                c2 = c2ref[0]
                et = ets.pop(h)
                for sqc in range(2):
                    e0 = 0 if sqc == 0 else 4
                    for sqt in range(4):
                        gq = sqc * 4 + sqt
                        cp = psp.tile([128, 72], F32, tag="ps",
                                      name=f"cp_{b}_{h}_{gq}")
                        for skt in range(gq + 1):
                            nc.tensor.matmul(cp[:, :65],
                                             et[:, e0 + skt,
                                                sqt * 128:(sqt + 1) * 128],
                                             vt[:, skt, :65],
                                             start=(skt == 0), stop=(skt == gq))
                        recd = small.tile([128, 1], F32, tag="recd")
                        nc.vector.reciprocal(recd[:], cp[:, 64:65])
                        nc.vector.tensor_scalar_mul(c2[:, gq, qb_:qb_ + 64],
                                                    cp[:, :64], recd[:])
                if h % 2 == 1 or h == 8:
                    for gq in range(8):
                        pt = psp.tile([128, 128], BF16, tag="ps",
                                      name=f"pt_{b}_{h}_{gq}")
                        nc.tensor.transpose(pt[:], c2[:, gq, :], idn[:])
                        nc.vector.tensor_copy(
                            cgc[:, h // 2, gq * 128:(gq + 1) * 128], pt[:])

            # stage 1: m0..2 k-outer so the PE consumes the rstd-scale
            # fill as it is produced (both token chunks, 6 psum banks)
            NS1 = 3
            whalf = {}
            for m in range(NS1):
                whalf[m] = wpp.tile([128, KT // 2, 2, 128], FP8, tag="wt",
                                    name=f"wha_{b}_{m}")
                nc.sync.dma_start(whalf[m][:], wpk[:, m, 0:KT // 2, :, :])
            ps1 = {}
            for m in range(NS1):
                for hc in range(2):
                    ps1[m, hc] = psp.tile([128, 512], F32, tag="ps",
                                          name=f"pb_{b}_{m}_{hc}")
            for kp in range(KT // 2):
                if kp == KT // 4:
                    for m in range(NS1):
                        whalf[m] = wpp.tile([128, KT // 2, 2, 128], FP8,
                                            tag="wt", name=f"whb_{b}_{m}")
                        nc.sync.dma_start(whalf[m][:],
                                          wpk[:, m, KT // 2:KT, :, :])
                k2 = slice(2 * kp, 2 * kp + 2)
                w2 = slice(2 * kp - (0 if kp < KT // 4 else KT // 2),
                           2 * kp + 2 - (0 if kp < KT // 4 else KT // 2))
                for m in range(NS1):
                    for hc in range(2):
                        tcols = slice(hc * 512, hc * 512 + 512)
                        nc.tensor.matmul(ps1[m, hc][:], whalf[m][:, w2, 0, :],
                                         xhl[:, k2, 0, tcols],
                                         start=(kp == 0), stop=False,
                                         perf_mode=DR)
                        nc.tensor.matmul(ps1[m, hc][:], whalf[m][:, w2, 1, :],
                                         xhl[:, k2, 0, tcols],
                                         start=False, stop=False, perf_mode=DR)
                        nc.tensor.matmul(ps1[m, hc][:], whalf[m][:, w2, 0, :],
                                         xhl[:, k2, 1, tcols],
                                         start=False,
                                         stop=(kp == KT // 2 - 1),
                                         perf_mode=DR)
            for m in range(NS1):
                for hc in range(2):
                    tcols = slice(hc * 512, hc * 512 + 512)
                    nc.vector.tensor_copy(qt[:, m, tcols], ps1[m, hc][:])
                rot = t2k.tile([128, S], BF16, tag="rot")
                tgt = qt[:, m, :]
                nc.vector.tensor_scalar_mul(rot[0:32, :], tgt[32:64, :], -1.0)
                nc.vector.tensor_copy(rot[32:64, :], tgt[0:32, :])
                nc.vector.tensor_scalar_mul(rot[64:96, :], tgt[96:128, :], -1.0)
                nc.vector.tensor_copy(rot[96:128, :], tgt[64:96, :])
                nc.vector.tensor_mul(tgt, tgt, cos_sb[:])
                nc.vector.tensor_mul(rot[:], rot[:], sin_sb[:])
                nc.vector.tensor_add(tgt, tgt, rot[:])

            for m in range(NS1, MT):
                wha = wpp.tile([128, KT // 2, 2, 128], FP8, tag="wt",
                               name=f"wha_{b}_{m}")
                nc.sync.dma_start(wha[:], wpk[:, m, 0:KT // 2, :, :])
                whb = wpp.tile([128, KT // 2, 2, 128], FP8, tag="wt",
                               name=f"whb_{b}_{m}")
                nc.sync.dma_start(whb[:], wpk[:, m, KT // 2:KT, :, :])
                for hc in range(2):
                    tcols = slice(hc * 512, hc * 512 + 512)
                    ps = psp.tile([128, 512], F32, tag="ps",
                                  name=f"pb_{b}_{m}_{hc}")
                    for kp in range(KT // 2):
                        k2 = slice(2 * kp, 2 * kp + 2)
                        wt = wha if kp < KT // 4 else whb
                        w2 = slice(2 * kp - (0 if kp < KT // 4 else KT // 2),
                                   2 * kp + 2 - (0 if kp < KT // 4 else KT // 2))
                        st_ = (kp == 0)
                        nc.tensor.matmul(ps[:], wt[:, w2, 0, :],
                                         xhl[:, k2, 0, tcols],
                                         start=st_, stop=False, perf_mode=DR)
                        nc.tensor.matmul(ps[:], wt[:, w2, 1, :],
                                         xhl[:, k2, 0, tcols],
                                         start=False, stop=False, perf_mode=DR)
                        nc.tensor.matmul(ps[:], wt[:, w2, 0, :],
                                         xhl[:, k2, 1, tcols],
                                         start=False, stop=(kp == KT // 2 - 1),
                                         perf_mode=DR)
                    if m < 5:
                        nc.vector.tensor_copy(qt[:, m, tcols], ps[:])
                    elif m == 5:
                        nc.vector.tensor_copy(kt2[0:64, tcols], ps[0:64, :])
                        vtmp = vtm.tile([64, 512], BF16, tag="vtmp")
                        nc.vector.tensor_copy(vtmp[:], ps[64:128, :])
                        for j in range(4):
                            pv = psp.tile([128, 64], BF16, tag="ps",
                                          name=f"pv_{b}_{hc}_{j}")
                            nc.tensor.transpose(
                                pv[:], vtmp[:, j * 128:(j + 1) * 128],
                                idn[0:64, 0:64])
                            nc.scalar.activation(vt[:, hc * 4 + j, 0:64],
                                                 pv[:], AF.Copy,
                                                 scale=1.0 / 16.0)
                    else:
                        gt_ = gtm.tile([128, 512], BF16, tag="gt")
                        nc.scalar.activation(gt_[:], ps[:], AF.Gelu,
                                             scale=1.0 / 16.0)
                        nc.scalar.activation(cgg[:, m - 6, 0, tcols], gt_[:],
                                             AF.Copy)
                        nc.gpsimd.tensor_tensor(cgg[:, m - 6, 1, tcols],
                                                gt_[:], cgg[:, m - 6, 0, tcols],
                                                op=SUB)
                if m < 5:
                    # rope both head slots of this m-tile, in place
                    rot = t2k.tile([128, S], BF16, tag="rot")
                    tgt = qt[:, m, :]
                    nc.vector.tensor_scalar_mul(rot[0:32, :], tgt[32:64, :], -1.0)
                    nc.vector.tensor_copy(rot[32:64, :], tgt[0:32, :])
                    nc.vector.tensor_scalar_mul(rot[64:96, :], tgt[96:128, :], -1.0)
                    nc.vector.tensor_copy(rot[96:128, :], tgt[64:96, :])
                    nc.vector.tensor_mul(tgt, tgt, cos_sb[:])
                    nc.vector.tensor_mul(rot[:], rot[:], sin_sb[:])
                    nc.vector.tensor_add(tgt, tgt, rot[:])
                elif m == 5:
                    rot = t2k.tile([128, S], BF16, tag="rot")
                    tgt = kt2[0:64, :]
                    nc.vector.tensor_scalar_mul(rot[0:32, :], kt2[32:64, :], -1.0)
                    nc.vector.tensor_copy(rot[32:64, :], kt2[0:32, :])
                    nc.vector.tensor_mul(tgt, tgt, cos_sb[0:64, :])
                    nc.vector.tensor_mul(rot[0:64, :], rot[0:64, :],
                                         sin_sb[0:64, :])
                    nc.vector.tensor_add(tgt, tgt, rot[0:64, :])
                    nc.vector.tensor_copy(kt2[64:128, :], kt2[0:64, :])
                if m in (10, 14, 18, 22):
                    emit_ctx((m - 10) // 2)
                elif m in (11, 15, 19, 23):
                    emit_ctx((m - 11) // 2 + 1)
                if m in (7, 11, 15, 19):
                    emit_scores((m - 7) // 2)
                    emit_scores((m - 7) // 2 + 1)
                elif m == 23:
                    emit_scores(8)

            emit_ctx(8)

        def emit_D(b):
            cgg = cgg_by[b]
            # ================= Phase D: dense + down -> DRAM ===============
            for hf in range(NHF):
                wq4 = []
                for q in range(4):
                    wqt = wdtp.tile([128, 12, 256], FP8, tag="wd",
                                    name=f"wd_{b}_{hf}_{q}")
                    nc.sync.dma_start(wqt[:], wdd[:, hf, q, :, :])
                    wq4.append(wqt)
                fcols = slice(hf * 256, hf * 256 + 256)
                pds = [psp.tile([128, 256], F32, tag="ps",
                                name=f"pd_{b}_{hf}_{r}") for r in range(8)]
                for kp in range(3):
                    for r in range(8):
                        k2 = slice(2 * kp, 2 * kp + 2)
                        tcols = slice(r * 128, r * 128 + 128)
                        nc.tensor.matmul(pds[r][:], cgc[:, k2, tcols],
                                         wq4[0][:, 4 * kp:4 * kp + 2, :],
                                         start=(kp == 0), stop=False,
                                         perf_mode=DR)
                        nc.tensor.matmul(pds[r][:], cgc[:, k2, tcols],
                                         wq4[0][:, 4 * kp + 2:4 * kp + 4, :],
                                         start=False, stop=False, perf_mode=DR)
                for kp in range(9):
                    q = 1 + kp // 3
                    j = kp % 3
                    k2 = slice(2 * kp, 2 * kp + 2)
                    for r in range(8):
                        tcols = slice(r * 128, r * 128 + 128)
                        whi = wq4[q][:, 4 * j:4 * j + 2, :]
                        wlo = wq4[q][:, 4 * j + 2:4 * j + 4, :]
                        nc.tensor.matmul(pds[r][:], cgg[:, k2, 0, tcols], whi,
                                         start=False, stop=False, perf_mode=DR)
                        nc.tensor.matmul(pds[r][:], cgg[:, k2, 0, tcols], wlo,
                                         start=False, stop=False, perf_mode=DR)
                        nc.tensor.matmul(pds[r][:], cgg[:, k2, 1, tcols], whi,
                                         start=False, stop=(kp == 8),
                                         perf_mode=DR)
                        if kp == 8:
                            ob = obp.tile([128, 256], BF16, tag="ob")
                            nc.scalar.activation(ob[:], pds[r][:], AF.Copy)
                            nc.sync.dma_start(
                                out[b * S + r * 128:b * S + (r + 1) * 128,
                                    fcols], ob[:])

        emit_A(0)
        emit_BC(0)
        emit_A(1)
        emit_D(0)
        emit_BC(1)
        emit_D(1)
    nc.compile()
    return nc


def _prep_inputs(hidden_states, cos, sin, ln_w1, ln_b1, ln_w2, ln_b2,
                 wq, wk, wv, w_dense, w_h4h, w_4hh):
    f32 = np.float32
    bf = ml_dtypes.bfloat16
    f8 = ml_dtypes.float8_e4m3fn
    lnw = np.concatenate([np.asarray(ln_w1), np.asarray(ln_w2)]).astype(np.float64)
    lnb = np.concatenate([np.asarray(ln_b1), np.asarray(ln_b2)]).astype(np.float64)

    X = np.asarray(hidden_states, f32).reshape(T, H).astype(bf)
    xbd = np.ascontiguousarray(X.astype(f8))             # [T, H] fp8 (stats)
    xtf = np.zeros((HP, T), bf)
    xtf[:H] = X.T
    xtd = np.ascontiguousarray(
        xtf.reshape(KT, 128, T).transpose(1, 0, 2))      # [128, KT, T]

    def pack16(W):
        # W [O, H] -> ln-folded, bias + colsum/16 rows, x16: [O, HP] f32
        W64 = W.astype(np.float64) * 16.0
        out_ = np.zeros((W.shape[0], HP), np.float64)
        out_[:, :H] = W64 * lnw
        out_[:, H] = W64 @ lnb
        out_[:, H + 1] = out_[:, :H].sum(1) / 16.0
        return out_.astype(f32)

    def hilo(Wp):
        hi = Wp.astype(f8)
        lo = (Wp - hi.astype(f32)).astype(f8)
        return hi, lo

    wq_f = np.asarray(wq, f32)          # [NH*HD, H]
    wk_f = np.asarray(wk, f32)
    wv_f = np.asarray(wv, f32)
    w14 = np.asarray(w_h4h, f32)        # [F4, H]
    wdT = np.asarray(w_dense, f32).T    # [NH*HD, H]
    w41T = np.asarray(w_4hh, f32).T     # [F4, H]

    cos2 = np.asarray(cos, f32)[0, 0] / 16.0   # [S, 64]
    sin2 = np.asarray(sin, f32)[0, 0] / 16.0
    csn = np.zeros((2, 128, S), bf)
    csn[0] = np.tile(cos2.T, (2, 1)).astype(bf)
    csn[1] = np.tile(sin2.T, (2, 1)).astype(bf)
    dmk = np.where(np.arange(128)[:, None] <= np.arange(128)[None, :],
                   0.0, NEGM).astype(f32)
    idn = np.eye(128, dtype=bf)
    seld = np.zeros((8, 1024), f8)
    for rt in range(8):
        seld[rt, rt * 128:(rt + 1) * 128] = 1.0

    in_maps = []
    for c in range(8):
        # --- projection weights [O=3072 rows, HP] ---
        Wall = np.zeros((MT * 128, H), f32)
        for s in range(10):
            gh = c * NSLOT + s
            if s < NSLOT and gh < NH:
                Wall[s * 64:(s + 1) * 64] = wq_f[gh * HD:(gh + 1) * HD]
        Wall[5 * 128:5 * 128 + 64] = wk_f
        Wall[5 * 128 + 64:6 * 128] = wv_f
        f0 = c * F4C
        Wall[6 * 128:6 * 128 + F4C] = w14[f0:f0 + F4C]
        Wp = pack16(Wall)
        hi, lo = hilo(Wp)
        # [O, HP] -> [128(p), MT, KT, 2, 128(j)]
        def swz(a):
            return a.reshape(MT, 128, KT, 128).transpose(3, 0, 2, 1)
        wpk = np.ascontiguousarray(
            np.stack([swz(hi), swz(lo)], axis=3))        # [128,MT,KT,2,128]

        # --- phase-D weights ---
        # dense rows laid out in ct order: feature f = pair*128 + within,
        # slot = pair*2 + (within>=64), d = within%64
        Wd = np.zeros((CKT * 128, HP), f32)
        for s in range(NSLOT):
            gh = c * NSLOT + s
            if gh >= NH:
                continue
            pair, half = divmod(s, 2)
            Wd[pair * 128 + half * 64:pair * 128 + half * 64 + 64, :H] = \
                (wdT[gh * HD:(gh + 1) * HD] * 16.0)
        W4 = np.zeros((GKT * 128, HP), f32)
        W4[:F4C, :H] = w41T[f0:f0 + F4C] * 16.0
        dh, dl = hilo(Wd)
        gh_, gl_ = hilo(W4)

        wddc = np.zeros((128, NHF, 4, 12, 256), f8)
        for hf in range(NHF):
            cols = slice(hf * 256, hf * 256 + 256)
            def kt_rows(a, k):
                return a[k * 128:(k + 1) * 128, cols]    # [128, 256]
            # Q0: dense pairs (hi01 lo01 hi23 lo23 hi45 lo45)
            for kp in range(3):
                wddc[:, hf, 0, 4 * kp + 0] = kt_rows(dh, 2 * kp)
                wddc[:, hf, 0, 4 * kp + 1] = kt_rows(dh, 2 * kp + 1)
                wddc[:, hf, 0, 4 * kp + 2] = kt_rows(dl, 2 * kp)
                wddc[:, hf, 0, 4 * kp + 3] = kt_rows(dl, 2 * kp + 1)
            # Q1..Q3: down kp triples
            for kp in range(9):
                q = 1 + kp // 3
                j = kp % 3
                wddc[:, hf, q, 4 * j + 0] = kt_rows(gh_, 2 * kp)
                wddc[:, hf, q, 4 * j + 1] = kt_rows(gh_, 2 * kp + 1)
                wddc[:, hf, q, 4 * j + 2] = kt_rows(gl_, 2 * kp)
                wddc[:, hf, q, 4 * j + 3] = kt_rows(gl_, 2 * kp + 1)
        in_maps.append({
            "xtd": xtd, "xbd": xbd, "wpk": wpk.astype(f8),
            "wdd": wddc, "csn": csn, "dmk": dmk, "idn": idn, "seld": seld,
        })
    return in_maps


def kernel(hidden_states, attention_mask, cos, sin,
           ln_w1, ln_b1, ln_w2, ln_b2,
           wq, wk, wv, w_dense, w_h4h, w_4hh):
    if "nc" not in _CACHE:
        _CACHE["nc"] = _build()
    nc = _CACHE["nc"]
    in_maps = _prep_inputs(hidden_states, cos, sin, ln_w1, ln_b1, ln_w2, ln_b2,
                           wq, wk, wv, w_dense, w_h4h, w_4hh)
    res = run_bass_kernel_spmd(nc, in_maps, core_ids=list(range(8)))
    acc = np.zeros((T, H), np.float64)
    for r in res.results:
        acc += r["out"][:, :H].astype(np.float64)
    outv = (acc / 16.0).astype(np.float32) \
        + np.asarray(hidden_states, np.float32).reshape(T, H)
    return outv.reshape(B, S, H).astype(np.float32)
